# revision 22
# baseline (speedup 1.0000x reference)
"""Tensor-parallel Llama MHA kernel for 8 TRN2 NeuronCores.

Problem: B=2, S=2048, HIDDEN=2048, 16 heads x head_dim 128, fp32, RoPE + causal.

Sharding: 8 cores = 2 (batch) x 4 (head groups of 4 heads).  Each core computes
q/k/v projections for its 4 heads, flash-style causal attention, and a partial
o_proj (attn_out_heads @ Wo[:, heads].T).  The full output is the sum of the 4
head-group partials per batch element, done on the host after gather.

Device kernel design (per core):
  - Matmul operands in bf16 (full 1 col/cycle PE rate; fp32/fp32r stream at
    half rate), fp32 PSUM accumulation, fp32 output.
  - All inputs are pre-swizzled on the host into the exact SBUF layout
    ([128 partitions, flat free dim]) so every DMA is per-partition
    contiguous.  x is seq-chunk-major so the first projection chunk only
    needs 2MB of x before the PE can start.
  - x.T fully SBUF-resident in bf16; weight half-panels double buffered;
    q-projection weights loaded first so the PE starts early.
  - RoPE: rotate_half via one extra 128-contraction matmul against a constant
    permutation matrix; combined with cos/sin on DVE.  The 1/sqrt(d) score
    scale is folded into the exp activation.
  - Attention (per head, per 512-query chunk): S.T blocks [k=128, q=512] for
    sub-diagonal key blocks; the diagonal 512x512 is computed as a packed
    trapezoid (widths 512/384/256/128) so upper-triangle work is skipped.
    Software-pipelined one unit ahead of the exp->mask->AV chain; exp on
    ScalarE (PSUM->SBUF bf16); triangular masks (128x128) on DVE; O.T [d, q]
    and column-sums l accumulated in PSUM (ones-matrix matmul for l);
    normalize by 1/l via the fast approximate DVE reciprocal.
  - o_proj lags one query chunk behind attention; its PSUM evacuations run
    on the otherwise-idle Pool engine (gpsimd) so DVE stays free.
  - Post pass: TRN2 instructions carry at most one sync wait; excess waits
    are peeled onto same-engine event-semaphore instructions.
"""

import math

import numpy as np

HIDDEN = 2048
NUM_HEADS = 16
HEAD_DIM = 128
BATCH = 2
SEQ = 2048
ROPE_BASE = 10000.0

N_CORES = 8
N_HGROUPS = N_CORES // BATCH          # 4 head-groups
H_LOCAL = NUM_HEADS // N_HGROUPS      # 4 heads per core
D = HEAD_DIM                          # 128
SQ = 512                              # query chunk (free dim of S.T blocks)
KB = 128                              # key block (partition dim of S.T blocks)


def build_bass(seq=SEQ, hid=HIDDEN, h_local=H_LOCAL):
    """Build the single-core Bass program (SPMD: same program on all cores)."""
    import concourse.bass as bass
    import concourse.tile as tile
    from concourse import mybir

    f32 = mybir.dt.float32
    bf16 = mybir.dt.bfloat16
    u32 = mybir.dt.uint32
    EXP = mybir.ActivationFunctionType.Exp
    MUL = mybir.AluOpType.mult
    SUB = mybir.AluOpType.subtract
    BYP = mybir.AluOpType.bypass

    n_qc = seq // SQ                  # query chunks
    n_kc = hid // 128                 # hidden (contraction) chunks
    n_sc = seq // SQ                  # seq chunks of 512
    n_ms = SQ // 128                  # 128-row subchunks in a 512 chunk
    n_on = hid // SQ                  # output col chunks of 512
    m_local = h_local                 # one M-chunk of 128 per head (d=128)
    M = h_local * D                   # projection output width
    isqrt_d = 1.0 / math.sqrt(D)

    nc = bass.Bass(target_bir_lowering=False, trn_type="TRN2")

    # ---- DRAM I/O: host pre-swizzled to [128, flat] layouts, bf16 ----
    xS = nc.dram_tensor("xS", [128, n_sc * n_kc * SQ], bf16, kind="ExternalInput")
    wqS = nc.dram_tensor("wqS", [128, n_kc * M], bf16, kind="ExternalInput")
    wkS = nc.dram_tensor("wkS", [128, n_kc * M], bf16, kind="ExternalInput")
    wvS = nc.dram_tensor("wvS", [128, n_kc * M], bf16, kind="ExternalInput")
    woS = nc.dram_tensor("woS", [128, h_local * hid], bf16, kind="ExternalInput")
    cosT = nc.dram_tensor("cosT", [D, seq], bf16, kind="ExternalInput")
    sinT = nc.dram_tensor("sinT", [D, seq], bf16, kind="ExternalInput")
    rotT = nc.dram_tensor("rotT", [D, D], bf16, kind="ExternalInput")
    maskS = nc.dram_tensor("maskS", [128, 2 * KB], bf16, kind="ExternalInput")
    onesd = nc.dram_tensor("ones", [128, 128], bf16, kind="ExternalInput")
    out = nc.dram_tensor("out", [seq, hid], bf16, kind="ExternalOutput")

    with tile.TileContext(nc) as tc:
        with (
            tc.tile_pool(name="persist", bufs=1) as persist,
            tc.tile_pool(name="psum", bufs=1, space="PSUM") as psum,
        ):
            # persistent SBUF tensors
            q_sb = persist.tile([128, h_local, seq], bf16)    # [d, head, seq]
            k_sb = persist.tile([128, h_local, seq], bf16)    # [d, head, seq]
            v_sb = persist.tile([128, seq // 128, M], bf16)   # [s%128, schunk, h*d]
            ones_sb = persist.tile([128, 128], bf16)
            m_sb = persist.tile([128, 2, KB], bf16)           # two triangle masks
            # constant for the magic-number reciprocal seed:
            # y0 = bits_to_f32(0x7EF311C3 - f32_to_bits(l)).  The uint ALU
            # saturates instead of wrapping, so compute C - bits via
            # scalar_tensor_tensor(bypass, subtract) -- bits(l) < C always.
            magic_sb = persist.tile([128, SQ], u32)
            nc.vector.memset(magic_sb, 0x7EF311C3)

            # ================= Phase 1-2: projections + RoPE =================
            with tc.tile_pool(name="proj", bufs=1) as proj:
                n_half = n_kc // 2  # hidden chunks per W half-panel

                def dma_w(w_half, w_dram, half, kc_lo, kc_hi):
                    nc.sync.dma_start(
                        out=w_half[:, kc_lo:kc_hi],
                        in_=w_dram[
                            :,
                            (half * n_half + kc_lo) * M
                            : (half * n_half + kc_hi) * M,
                        ].rearrange("p (kc m) -> p kc m", m=M),
                    )

                def load_w_halves(w_dram):
                    halves = []
                    for half in range(2):
                        w_half = proj.tile(
                            [128, n_half, M], bf16, tag="w_half", bufs=2
                        )
                        dma_w(w_half, w_dram, half, 0, n_half)
                        halves.append(w_half)
                    return halves

                def dma_x(x_res, n, kc_lo, kc_hi):
                    nc.sync.dma_start(
                        out=x_res[:, n, kc_lo:kc_hi],
                        in_=xS[
                            :,
                            (n * n_kc + kc_lo) * SQ : (n * n_kc + kc_hi) * SQ,
                        ].rearrange("p (kc s) -> p kc s", s=SQ),
                    )

                # DMAs on one HWDGE queue complete roughly in issue order, so
                # front-load exactly what the first matmuls need: the first
                # kc chunks of wq half 0 and of x seq-chunk 0.
                x_res = proj.tile([128, n_sc, n_kc, SQ], bf16)
                wq_h0 = proj.tile([128, n_half, M], bf16, tag="w_half", bufs=2)
                dma_w(wq_h0, wqS, 0, 0, 2)
                dma_x(x_res, 0, 0, 4)
                dma_w(wq_h0, wqS, 0, 2, n_half)
                dma_x(x_res, 0, 4, n_kc)
                wq_h1 = proj.tile([128, n_half, M], bf16, tag="w_half", bufs=2)
                dma_w(wq_h1, wqS, 1, 0, n_half)
                for n in range(1, n_sc):
                    dma_x(x_res, n, 0, n_kc)
                w_q_halves = [wq_h0, wq_h1]

                # small constants on the Activation HWDGE queue so they don't
                # queue behind the bulk x/w transfers
                cos_sb = proj.tile([128, seq], bf16)
                sin_sb = proj.tile([128, seq], bf16)
                rot_sb = proj.tile([128, 128], bf16)
                nc.scalar.dma_start(out=rot_sb, in_=rotT[:])
                nc.scalar.dma_start(out=cos_sb, in_=cosT[:])
                nc.scalar.dma_start(out=sin_sb, in_=sinT[:])
                nc.scalar.dma_start(out=ones_sb, in_=onesd[:])
                nc.scalar.dma_start(
                    out=m_sb,
                    in_=maskS[:].rearrange("p (j q) -> p j q", q=KB),
                )

                for proj_i, (w_dram, dst, is_v) in enumerate(
                    (
                        (wqS, q_sb, False),
                        (wkS, k_sb, False),
                        (wvS, v_sb, True),
                    )
                ):
                    w_halves = w_q_halves if proj_i == 0 else load_w_halves(w_dram)

                    for n in range(n_sc):
                        # PSUM accumulators for this seq chunk, bank-aligned
                        # (one accumulation group per 2KB PSUM bank)
                        n_acc = ((n_ms if is_v else m_local) * 512) // 1024
                        ps = []
                        for t in range(n_acc):
                            ps_t = psum.tile([128, 1024], f32, tag="s", bufs=2)
                            ps.append(ps_t)

                        def acc_slice(i, width):
                            return ps[(i * 512) // 1024][
                                :, (i * 512) % 1024 : (i * 512) % 1024 + width
                            ]

                        for half in range(2):
                            w_half = w_halves[half]
                            for kc in range(n_half):
                                kc_g = half * n_half + kc
                                x_t = x_res[:, n, kc_g, :]
                                start = kc_g == 0
                                stop = kc_g == n_kc - 1
                                if not is_v:
                                    for m in range(m_local):
                                        nc.tensor.matmul(
                                            acc_slice(m, SQ),
                                            lhsT=w_half[:, kc, m * D : (m + 1) * D],
                                            rhs=x_t,
                                            start=start,
                                            stop=stop,
                                        )
                                else:
                                    for sub in range(n_ms):
                                        nc.tensor.matmul(
                                            acc_slice(sub, M),
                                            lhsT=x_res[
                                                :, n, kc_g,
                                                sub * 128 : (sub + 1) * 128,
                                            ],
                                            rhs=w_half[:, kc, :],
                                            start=start,
                                            stop=stop,
                                        )
                        if is_v:
                            # split evacuation across ScalarE and VectorE so
                            # the PSUM slots free in half the time
                            for sub in range(n_ms):
                                if sub % 2 == 0:
                                    nc.scalar.copy(
                                        out=v_sb[:, n * n_ms + sub, :],
                                        in_=acc_slice(sub, M),
                                    )
                                else:
                                    nc.vector.tensor_copy(
                                        v_sb[:, n * n_ms + sub, :],
                                        acc_slice(sub, M),
                                    )
                        else:
                            # RoPE for the heads of this seq chunk
                            for t in range(n_acc):
                                qraw = proj.tile(
                                    [128, 1024], bf16, tag="qraw", bufs=2
                                )
                                if t % 2 == 0:
                                    nc.scalar.copy(out=qraw, in_=ps[t])
                                else:
                                    nc.vector.tensor_copy(qraw, ps[t])
                                for p in range(2):
                                    m = 2 * t + p
                                    rh = psum.tile([128, 512], f32, tag="o", bufs=4)
                                    nc.tensor.matmul(
                                        rh,
                                        lhsT=rot_sb,
                                        rhs=qraw[:, p * 512 : (p + 1) * 512],
                                        start=True,
                                        stop=True,
                                    )
                                    dstv = dst[:, m, n * SQ : (n + 1) * SQ]
                                    tmp = proj.tile(
                                        [128, 512], bf16, tag="tmp", bufs=3
                                    )
                                    nc.vector.tensor_mul(
                                        tmp, rh, sin_sb[:, n * SQ : (n + 1) * SQ]
                                    )
                                    nc.vector.tensor_mul(
                                        dstv,
                                        qraw[:, p * 512 : (p + 1) * 512],
                                        cos_sb[:, n * SQ : (n + 1) * SQ],
                                    )
                                    nc.vector.tensor_add(dstv, dstv, tmp)

            # ================= Phase 3: attention + o_proj =================
            with tc.tile_pool(name="attn", bufs=1) as attn:
                wo_sb = attn.tile([128, h_local, hid], bf16)  # [d, head, hid]
                nc.scalar.dma_start(
                    out=wo_sb,
                    in_=woS[:].rearrange("p (h n) -> p h n", n=hid),
                )

                pend_ot = {}

                def o_proj_block(qc, ms, on, ring="o"):
                    # one [128q, 512] block of o_proj for query chunk qc.
                    # Normally allocated from the "o" ring right after a
                    # normalize freed o_ps/l_ps (deadlock-free there); the
                    # trailing blocks alternate rings so PSUM evacuation
                    # never serializes the PE.
                    if ring == "o":
                        oo0 = psum.tile([128, SQ], f32, tag="o", bufs=4)
                    else:
                        oo = psum.tile([128, 2, 512], f32, tag="s", bufs=2)
                        oo0 = oo[:, 0, :]
                    for h in range(h_local):
                        nc.tensor.matmul(
                            oo0,
                            lhsT=pend_ot[(qc, h)][:, ms * 128 : (ms + 1) * 128],
                            rhs=wo_sb[:, h, on * SQ : (on + 1) * SQ],
                            start=(h == 0),
                            stop=(h == h_local - 1),
                        )
                    # bf16 output halves evac + DMA cost; evacuate on DVE --
                    # ScalarE is saturated by exp in the surrounding chunk
                    out_t = attn.tile([128, SQ], bf16, tag="out_t", bufs=8)
                    nc.vector.tensor_copy(out_t, oo0)
                    nc.sync.dma_start(
                        out=out[
                            qc * SQ + ms * 128 : qc * SQ + (ms + 1) * 128,
                            on * SQ : (on + 1) * SQ,
                        ],
                        in_=out_t,
                    )

                # Software pipeline: the exp for a unit (pair of full k-blocks,
                # or half the diagonal trapezoid) is emitted right after its S
                # matmuls so ScalarE starts immediately; mask/AV/l consumption
                # runs one unit behind, so the PE always has the next unit's S
                # matmuls to chew on while the exp completes.
                def consume(st):
                    (segs, is_diag, p_sb, o_ps, l_ps, h, qc, n_kb) = st
                    if is_diag:
                        # triangular masks for the first 128 cols of each seg
                        nc.vector.tensor_mul(
                            p_sb[:, :, 0:KB], p_sb[:, :, 0:KB], m_sb
                        )
                    for plane, kb, w, qoff in segs:
                        rhs = p_sb[:, plane, 0:w]
                        start = kb == 0
                        stop = kb == n_kb - 1
                        nc.tensor.matmul(
                            o_ps[:, qoff:SQ],
                            lhsT=v_sb[:, kb, h * D : (h + 1) * D],
                            rhs=rhs,
                            start=start,
                            stop=stop,
                            skip_group_check=True,
                        )
                        nc.tensor.matmul(
                            l_ps[:, qoff:SQ],
                            lhsT=ones_sb,
                            rhs=rhs,
                            start=start,
                            stop=stop,
                            skip_group_check=True,
                        )
                    if segs[-1][1] == n_kb - 1:
                        # last unit of (h, qc): normalize ot = o_ps * (1/l),
                        # all on DVE (ScalarE has no throughput headroom):
                        # integer magic-number seed + one Newton step gives
                        # 1/l to ~0.1% -- plenty under the softmax noise.
                        y0 = attn.tile([128, SQ], f32, tag="linv", bufs=4)
                        nc.vector.scalar_tensor_tensor(
                            out=y0.bitcast(u32), in0=magic_sb, scalar=0.0,
                            in1=l_ps.bitcast(u32), op0=BYP, op1=SUB,
                        )
                        t = attn.tile([128, SQ], f32, tag="linv", bufs=4)
                        nc.vector.tensor_mul(t, l_ps, y0)
                        t2 = attn.tile([128, SQ], f32, tag="linv", bufs=4)
                        # -y1 = (l*y0 - 2) * y0
                        nc.vector.scalar_tensor_tensor(
                            out=t2, in0=t, scalar=2.0, in1=y0,
                            op0=SUB, op1=MUL,
                        )
                        ot = attn.tile([128, SQ], bf16, tag="ot", bufs=8)
                        # ot = (o_ps * -1) * (-y1)
                        nc.vector.scalar_tensor_tensor(
                            out=ot, in0=o_ps, scalar=-1.0, in1=t2,
                            op0=MUL, op1=MUL,
                        )
                        pend_ot[(qc, h)] = ot

                pend = None
                pending_oproj = []
                # reverse chunk order: the PE-rich qc=3 runs first (nothing
                # pending to interleave), and the ScalarE-bound qc=0 runs
                # last with o_proj(1)'s matmuls soaking up the PE slack
                for qc in reversed(range(n_qc)):
                    n_kb = (qc + 1) * (SQ // KB)  # causal: blocks 0..n_kb-1
                    kbB = 4 * qc                  # first diagonal block
                    n_units = h_local * (2 * qc + 2)
                    # spread the previous chunk's o_proj blocks evenly over
                    # this chunk's units so PE/DVE/DMA load stays smooth and
                    # ScalarE-bound units get free PE slack
                    emit_after = {
                        (i + 1) * n_units // len(pending_oproj): i + 1
                        for i in range(len(pending_oproj))
                    } if pending_oproj else {}
                    ui = 0
                    emitted = 0
                    for h in range(h_local):
                        o_ps = psum.tile([128, SQ], f32, tag="o", bufs=4)
                        l_ps = psum.tile([128, SQ], f32, tag="o", bufs=4)
                        # units: full pairs then the packed diagonal trapezoid
                        units = [
                            (False, [(p, 2 * kbp + p, 512, 0) for p in range(2)])
                            for kbp in range(2 * qc)
                        ]
                        units.append(
                            (True, [(0, kbB, 512, 0), (1, kbB + 1, 384, 128)])
                        )
                        units.append(
                            (True, [(0, kbB + 2, 256, 256), (1, kbB + 3, 128, 384)])
                        )
                        for is_diag, segs in units:
                            s_ps = psum.tile([128, 2, 512], f32, tag="s", bufs=2)
                            for plane, kb, w, qoff in segs:
                                nc.tensor.matmul(
                                    s_ps[:, plane, 0:w],
                                    lhsT=k_sb[:, h, kb * KB : (kb + 1) * KB],
                                    rhs=q_sb[:, h, qc * SQ + qoff : (qc + 1) * SQ],
                                    start=True,
                                    stop=True,
                                )
                            p_sb = attn.tile([128, 2, 512], bf16, tag="p_sb", bufs=4)
                            if segs[-1][2] <= 256:
                                # second diagonal half: only 2x256 cols live
                                nc.scalar.activation(
                                    p_sb[:, :, 0:256], s_ps[:, :, 0:256],
                                    EXP, scale=isqrt_d,
                                )
                            else:
                                nc.scalar.activation(p_sb, s_ps, EXP, scale=isqrt_d)
                            if pend is not None:
                                consume(pend)
                            pend = (segs, is_diag, p_sb, o_ps, l_ps, h, qc, n_kb)
                            ui += 1
                            want = emit_after.get(ui, emitted)
                            while emitted < want:
                                o_proj_block(*pending_oproj[emitted])
                                emitted += 1
                    # flush the pipeline at the chunk boundary
                    if pend is not None:
                        consume(pend)
                        pend = None
                    while emitted < len(pending_oproj):
                        o_proj_block(*pending_oproj[emitted])
                        emitted += 1
                    pending_oproj = [
                        (qc, ms, on)
                        for ms in range(n_ms)
                        for on in range(n_on)
                    ]
                # trailing o_proj for the last chunk
                for blk in pending_oproj:
                    o_proj_block(*blk)

    # Finalize (assigns semaphore waits), then legalize: TRN2 instructions
    # accept only ONE sync wait each (EventSemaphore: two, InstISA: zero).
    nc.to_json_bytes()
    _legalize_waits(nc, mybir)
    return nc


def _legalize_waits(nc, mybir):
    """TRN2 instructions carry at most ONE sync wait (InstEventSemaphore:
    two; raw InstISA: none).  Peel excess waits onto event-semaphore
    instructions inserted immediately before, on the same engine sequencer
    (program order keeps the semantics)."""
    nfix = 0
    for f in nc.m.functions:
        for blk in f.blocks:
            insts = list(blk.instructions)
            out = []
            changed = False
            for inst in insts:
                si = getattr(inst, "sync_info", None)
                waits = list(si.on_wait) if si is not None and si.on_wait else []
                tname = type(inst).__name__
                limit = 2 if tname == "InstEventSemaphore" else (
                    0 if tname == "InstISA" else 1
                )
                if len(waits) > limit:
                    keep, excess = waits[:limit], waits[limit:]
                    for k in range(0, len(excess), 2):
                        es = mybir.InstEventSemaphore(
                            name=f"I-waitfix-{nfix}", ins=[], outs=[]
                        )
                        nfix += 1
                        es.engine = inst.engine
                        es.sync_info = mybir.SyncInfo(
                            on_wait=list(excess[k : k + 2]), on_update=[]
                        )
                        nc.register_instruction(es)
                        out.append(es)
                    inst.sync_info = mybir.SyncInfo(
                        on_wait=keep, on_update=list(si.on_update or [])
                    )
                    changed = True
                out.append(inst)
            if changed:
                blk.instructions = out
    return nfix


# ---------------------------------------------------------------------------
# Host-side input prep
# ---------------------------------------------------------------------------

def _rope_cache_np(seq, d):
    inv_freq = 1.0 / (ROPE_BASE ** (np.arange(0, d, 2, dtype=np.float32) / d))
    pos = np.arange(seq, dtype=np.float32)
    rot = pos[:, None] * inv_freq[None, :].astype(np.float32)
    theta = np.concatenate([rot, rot], axis=-1)  # [s, d]
    return np.cos(theta).astype(np.float32), np.sin(theta).astype(np.float32)


def _rot_matrix_np(d):
    """lhsT for rotate_half: (rotT.T @ q) == rotate_half(q)."""
    h = d // 2
    RT = np.zeros((d, d), dtype=np.float32)
    RT[np.arange(h) + h, np.arange(h)] = -1.0  # out[i] = -q[i+h], i < h
    RT[np.arange(h), np.arange(h) + h] = 1.0   # out[i] = q[i-h],  i >= h
    return RT


def _tri_masks_np():
    """Two copies of the 128x128 lower-triangle mask: m[j, kk, qq] = kk <= qq."""
    kk = np.arange(KB)[None, :, None]
    qq = np.arange(KB)[None, None, :]
    tri = (kk <= qq).astype(np.float32)       # [1, 128, 128]
    return np.broadcast_to(tri, (2, KB, KB))


def _swizzle_kc(a2d):
    """[n_kc*128, F] -> [128, n_kc*F] (partition-contiguous SBUF layout)."""
    n_kc = a2d.shape[0] // 128
    return np.ascontiguousarray(
        a2d.reshape(n_kc, 128, a2d.shape[1]).transpose(1, 0, 2).reshape(128, -1)
    )


def _swizzle_x(a2d):
    """[n_kc*128, S] -> [128, n_sc, n_kc, 512] -> [128, flat] seq-chunk-major."""
    n_kc = a2d.shape[0] // 128
    n_sc = a2d.shape[1] // SQ
    return np.ascontiguousarray(
        a2d.reshape(n_kc, 128, n_sc, SQ)
        .transpose(1, 2, 0, 3)
        .reshape(128, -1)
    )


def make_in_maps(hidden_states, Wq, Wk, Wv, Wo):
    import ml_dtypes

    bf = ml_dtypes.bfloat16
    cos, sin = _rope_cache_np(SEQ, D)
    cosT = np.ascontiguousarray(cos.T).astype(bf)
    sinT = np.ascontiguousarray(sin.T).astype(bf)
    rotT = _rot_matrix_np(D).astype(bf)
    mask = _tri_masks_np()  # [2, 128, 128]
    maskS = np.ascontiguousarray(
        mask.transpose(1, 0, 2).reshape(128, -1)
    ).astype(bf)
    ones = np.ones((128, 128), dtype=bf)

    in_maps = []
    for core in range(N_CORES):
        b = core // N_HGROUPS
        g = core % N_HGROUPS
        rs = slice(g * H_LOCAL * D, (g + 1) * H_LOCAL * D)
        in_maps.append(
            {
                "xS": _swizzle_x(hidden_states[b].T).astype(bf),
                "wqS": _swizzle_kc(Wq[rs, :].T).astype(bf),
                "wkS": _swizzle_kc(Wk[rs, :].T).astype(bf),
                "wvS": _swizzle_kc(Wv[rs, :].T).astype(bf),
                "woS": _swizzle_kc(Wo[:, rs].T).astype(bf),
                "cosT": cosT,
                "sinT": sinT,
                "rotT": rotT,
                "maskS": maskS,
                "ones": ones,
            }
        )
    return in_maps


def combine_outputs(results):
    """results: list of 8 dicts with 'out' [SEQ, HIDDEN] -> [BATCH, SEQ, HIDDEN]."""
    out = np.zeros((BATCH, SEQ, HIDDEN), dtype=np.float32)
    for core, r in enumerate(results):
        b = core // N_HGROUPS
        out[b] += np.asarray(r["out"], dtype=np.float32)
    return out


_CACHE = {}


def run_hw(inputs, trace=False, **kw):
    """Run on 8 NeuronCores; returns (output, BassKernelResults)."""
    from concourse.bass_utils import run_bass_kernel_spmd

    if "nc" not in _CACHE:
        _CACHE["nc"] = build_bass()
    nc = _CACHE["nc"]
    in_maps = make_in_maps(
        np.asarray(inputs["hidden_states"], dtype=np.float32),
        np.asarray(inputs["Wq"], dtype=np.float32),
        np.asarray(inputs["Wk"], dtype=np.float32),
        np.asarray(inputs["Wv"], dtype=np.float32),
        np.asarray(inputs["Wo"], dtype=np.float32),
    )
    res = run_bass_kernel_spmd(
        nc, in_maps, core_ids=list(range(N_CORES)), trace=trace, **kw
    )
    return combine_outputs(res.results), res


def kernel(hidden_states, Wq, Wk, Wv, Wo):
    out, _ = run_hw(
        {
            "hidden_states": hidden_states,
            "Wq": Wq,
            "Wk": Wk,
            "Wv": Wv,
            "Wo": Wo,
        }
    )
    return out


# revision 27
# speedup vs baseline: 1.0335x; 1.0335x over previous
"""Tensor-parallel Llama MHA kernel for 8 TRN2 NeuronCores.

Problem: B=2, S=2048, HIDDEN=2048, 16 heads x head_dim 128, fp32, RoPE + causal.

Sharding: 8 cores = 2 (batch) x 4 (head groups of 4 heads).  Each core computes
q/k/v projections for its 4 heads, flash-style causal attention, and a partial
o_proj (attn_out_heads @ Wo[:, heads].T).  The full output is the sum of the 4
head-group partials per batch element, done on the host after gather.

Device kernel design (per core):
  - Matmul operands in bf16 (full 1 col/cycle PE rate; fp32/fp32r stream at
    half rate), fp32 PSUM accumulation, fp32 output.
  - All inputs are pre-swizzled on the host into the exact SBUF layout
    ([128 partitions, flat free dim]) so every DMA is per-partition
    contiguous.  x is seq-chunk-major so the first projection chunk only
    needs 2MB of x before the PE can start.
  - x.T fully SBUF-resident in bf16; weight half-panels double buffered;
    q-projection weights loaded first so the PE starts early.
  - RoPE: rotate_half via one extra 128-contraction matmul against a constant
    permutation matrix; combined with cos/sin on DVE.  The 1/sqrt(d) score
    scale is folded into the exp activation.
  - Attention (per head, per 512-query chunk): S.T blocks [k=128, q=512] for
    sub-diagonal key blocks; the diagonal 512x512 is computed as a packed
    trapezoid (widths 512/384/256/128) so upper-triangle work is skipped.
    Software-pipelined one unit ahead of the exp->mask->AV chain; exp on
    ScalarE (PSUM->SBUF bf16); triangular masks (128x128) on DVE; O.T [d, q]
    and column-sums l accumulated in PSUM (ones-matrix matmul for l);
    normalize by 1/l via the fast approximate DVE reciprocal.
  - o_proj lags one query chunk behind attention; its PSUM evacuations run
    on the otherwise-idle Pool engine (gpsimd) so DVE stays free.
  - Post pass: TRN2 instructions carry at most one sync wait; excess waits
    are peeled onto same-engine event-semaphore instructions.
"""

import math

import numpy as np

HIDDEN = 2048
NUM_HEADS = 16
HEAD_DIM = 128
BATCH = 2
SEQ = 2048
ROPE_BASE = 10000.0

N_CORES = 8
N_HGROUPS = N_CORES // BATCH          # 4 head-groups
H_LOCAL = NUM_HEADS // N_HGROUPS      # 4 heads per core
D = HEAD_DIM                          # 128
SQ = 512                              # query chunk (free dim of S.T blocks)
KB = 128                              # key block (partition dim of S.T blocks)


def build_bass(seq=SEQ, hid=HIDDEN, h_local=H_LOCAL):
    """Build the single-core Bass program (SPMD: same program on all cores)."""
    import concourse.bass as bass
    import concourse.tile as tile
    from concourse import mybir

    f32 = mybir.dt.float32
    bf16 = mybir.dt.bfloat16
    u32 = mybir.dt.uint32
    EXP = mybir.ActivationFunctionType.Exp
    MUL = mybir.AluOpType.mult
    SUB = mybir.AluOpType.subtract
    BYP = mybir.AluOpType.bypass

    n_qc = seq // SQ                  # query chunks
    n_kc = hid // 128                 # hidden (contraction) chunks
    n_sc = seq // SQ                  # seq chunks of 512
    n_ms = SQ // 128                  # 128-row subchunks in a 512 chunk
    n_on = hid // SQ                  # output col chunks of 512
    m_local = h_local                 # one M-chunk of 128 per head (d=128)
    M = h_local * D                   # projection output width
    isqrt_d = 1.0 / math.sqrt(D)

    nc = bass.Bass(target_bir_lowering=False, trn_type="TRN2")

    # ---- DRAM I/O: host pre-swizzled to [128, flat] layouts, bf16 ----
    xS = nc.dram_tensor("xS", [128, n_sc * n_kc * SQ], bf16, kind="ExternalInput")
    wqS = nc.dram_tensor("wqS", [128, n_kc * M], bf16, kind="ExternalInput")
    wkS = nc.dram_tensor("wkS", [128, n_kc * M], bf16, kind="ExternalInput")
    wvS = nc.dram_tensor("wvS", [128, n_kc * M], bf16, kind="ExternalInput")
    woS = nc.dram_tensor("woS", [128, h_local * hid], bf16, kind="ExternalInput")
    cosT = nc.dram_tensor("cosT", [D, seq], bf16, kind="ExternalInput")
    sinT = nc.dram_tensor("sinT", [D, seq], bf16, kind="ExternalInput")
    rotT = nc.dram_tensor("rotT", [D, D], bf16, kind="ExternalInput")
    maskS = nc.dram_tensor("maskS", [128, 2 * KB], bf16, kind="ExternalInput")
    onesd = nc.dram_tensor("ones", [128, 128], bf16, kind="ExternalInput")
    out = nc.dram_tensor("out", [seq, hid], bf16, kind="ExternalOutput")

    with tile.TileContext(nc) as tc:
        with (
            tc.tile_pool(name="persist", bufs=1) as persist,
            tc.tile_pool(name="psum", bufs=1, space="PSUM") as psum,
        ):
            # persistent SBUF tensors
            q_sb = persist.tile([128, h_local, seq], bf16)    # [d, head, seq]
            k_sb = persist.tile([128, h_local, seq], bf16)    # [d, head, seq]
            v_sb = persist.tile([128, seq // 128, M], bf16)   # [s%128, schunk, h*d]
            ones_sb = persist.tile([128, 128], bf16)
            m_sb = persist.tile([128, 2, KB], bf16)           # two triangle masks
            # constant for the magic-number reciprocal seed:
            # y0 = bits_to_f32(0x7EF311C3 - f32_to_bits(l)).  The uint ALU
            # saturates instead of wrapping, so compute C - bits via
            # scalar_tensor_tensor(bypass, subtract) -- bits(l) < C always.
            magic_sb = persist.tile([128, SQ], u32)
            nc.vector.memset(magic_sb, 0x7EF311C3)

            # ================= Phase 1-2: projections + RoPE =================
            with tc.tile_pool(name="proj", bufs=1) as proj:
                n_half = n_kc // 2  # hidden chunks per W half-panel

                def dma_w(w_half, w_dram, half, kc_lo, kc_hi):
                    nc.sync.dma_start(
                        out=w_half[:, kc_lo:kc_hi],
                        in_=w_dram[
                            :,
                            (half * n_half + kc_lo) * M
                            : (half * n_half + kc_hi) * M,
                        ].rearrange("p (kc m) -> p kc m", m=M),
                    )

                def load_w_halves(w_dram):
                    halves = []
                    for half in range(2):
                        w_half = proj.tile(
                            [128, n_half, M], bf16, tag="w_half", bufs=2
                        )
                        dma_w(w_half, w_dram, half, 0, n_half)
                        halves.append(w_half)
                    return halves

                def dma_x(x_res, n, kc_lo, kc_hi):
                    nc.sync.dma_start(
                        out=x_res[:, n, kc_lo:kc_hi],
                        in_=xS[
                            :,
                            (n * n_kc + kc_lo) * SQ : (n * n_kc + kc_hi) * SQ,
                        ].rearrange("p (kc s) -> p kc s", s=SQ),
                    )

                # DMAs on one HWDGE queue complete roughly in issue order, so
                # front-load exactly what the first matmuls need: the first
                # kc chunks of wq half 0 and of x seq-chunk 0.
                x_res = proj.tile([128, n_sc, n_kc, SQ], bf16)
                wq_h0 = proj.tile([128, n_half, M], bf16, tag="w_half", bufs=2)
                dma_w(wq_h0, wqS, 0, 0, 2)
                dma_x(x_res, 0, 0, 4)
                dma_w(wq_h0, wqS, 0, 2, n_half)
                dma_x(x_res, 0, 4, n_kc)
                wq_h1 = proj.tile([128, n_half, M], bf16, tag="w_half", bufs=2)
                dma_w(wq_h1, wqS, 1, 0, n_half)
                for n in range(1, n_sc):
                    dma_x(x_res, n, 0, n_kc)
                w_q_halves = [wq_h0, wq_h1]

                # small constants on the Activation HWDGE queue so they don't
                # queue behind the bulk x/w transfers
                cos_sb = proj.tile([128, seq], bf16)
                sin_sb = proj.tile([128, seq], bf16)
                rot_sb = proj.tile([128, 128], bf16)
                nc.scalar.dma_start(out=rot_sb, in_=rotT[:])
                nc.scalar.dma_start(out=cos_sb, in_=cosT[:])
                nc.scalar.dma_start(out=sin_sb, in_=sinT[:])
                nc.scalar.dma_start(out=ones_sb, in_=onesd[:])
                nc.scalar.dma_start(
                    out=m_sb,
                    in_=maskS[:].rearrange("p (j q) -> p j q", q=KB),
                )

                for proj_i, (w_dram, dst, is_v) in enumerate(
                    (
                        (wqS, q_sb, False),
                        (wkS, k_sb, False),
                        (wvS, v_sb, True),
                    )
                ):
                    w_halves = w_q_halves if proj_i == 0 else load_w_halves(w_dram)

                    for n in range(n_sc):
                        # PSUM accumulators for this seq chunk, bank-aligned
                        # (one accumulation group per 2KB PSUM bank)
                        n_acc = ((n_ms if is_v else m_local) * 512) // 1024
                        ps = []
                        for t in range(n_acc):
                            ps_t = psum.tile([128, 1024], f32, tag="s", bufs=2)
                            ps.append(ps_t)

                        def acc_slice(i, width):
                            return ps[(i * 512) // 1024][
                                :, (i * 512) % 1024 : (i * 512) % 1024 + width
                            ]

                        for half in range(2):
                            w_half = w_halves[half]
                            for kc in range(n_half):
                                kc_g = half * n_half + kc
                                x_t = x_res[:, n, kc_g, :]
                                start = kc_g == 0
                                stop = kc_g == n_kc - 1
                                if not is_v:
                                    for m in range(m_local):
                                        nc.tensor.matmul(
                                            acc_slice(m, SQ),
                                            lhsT=w_half[:, kc, m * D : (m + 1) * D],
                                            rhs=x_t,
                                            start=start,
                                            stop=stop,
                                        )
                                else:
                                    for sub in range(n_ms):
                                        nc.tensor.matmul(
                                            acc_slice(sub, M),
                                            lhsT=x_res[
                                                :, n, kc_g,
                                                sub * 128 : (sub + 1) * 128,
                                            ],
                                            rhs=w_half[:, kc, :],
                                            start=start,
                                            stop=stop,
                                        )
                        if is_v:
                            # split evacuation across ScalarE and VectorE so
                            # the PSUM slots free in half the time
                            for sub in range(n_ms):
                                if sub % 2 == 0:
                                    nc.scalar.copy(
                                        out=v_sb[:, n * n_ms + sub, :],
                                        in_=acc_slice(sub, M),
                                    )
                                else:
                                    nc.vector.tensor_copy(
                                        v_sb[:, n * n_ms + sub, :],
                                        acc_slice(sub, M),
                                    )
                        else:
                            # RoPE for the heads of this seq chunk
                            for t in range(n_acc):
                                qraw = proj.tile(
                                    [128, 1024], bf16, tag="qraw", bufs=2
                                )
                                if t % 2 == 0:
                                    nc.scalar.copy(out=qraw, in_=ps[t])
                                else:
                                    nc.vector.tensor_copy(qraw, ps[t])
                                for p in range(2):
                                    m = 2 * t + p
                                    rh = psum.tile([128, 512], f32, tag="o", bufs=4)
                                    nc.tensor.matmul(
                                        rh,
                                        lhsT=rot_sb,
                                        rhs=qraw[:, p * 512 : (p + 1) * 512],
                                        start=True,
                                        stop=True,
                                    )
                                    dstv = dst[:, m, n * SQ : (n + 1) * SQ]
                                    tmp = proj.tile(
                                        [128, 512], bf16, tag="tmp", bufs=3
                                    )
                                    nc.vector.tensor_mul(
                                        tmp, rh, sin_sb[:, n * SQ : (n + 1) * SQ]
                                    )
                                    nc.vector.tensor_mul(
                                        dstv,
                                        qraw[:, p * 512 : (p + 1) * 512],
                                        cos_sb[:, n * SQ : (n + 1) * SQ],
                                    )
                                    nc.vector.tensor_add(dstv, dstv, tmp)

            # ================= Phase 3: attention + o_proj =================
            with tc.tile_pool(name="attn", bufs=1) as attn:
                wo_sb = attn.tile([128, h_local, hid], bf16)  # [d, head, hid]
                nc.scalar.dma_start(
                    out=wo_sb,
                    in_=woS[:].rearrange("p (h n) -> p h n", n=hid),
                )

                pend_ot = {}

                def o_proj_block(qc, ms, on, ring="o"):
                    # one [128q, 512] block of o_proj for query chunk qc.
                    # Normally allocated from the "o" ring right after a
                    # normalize freed o_ps/l_ps (deadlock-free there); the
                    # trailing blocks alternate rings so PSUM evacuation
                    # never serializes the PE.
                    if ring == "o":
                        oo0 = psum.tile([128, SQ], f32, tag="o", bufs=4)
                    else:
                        oo = psum.tile([128, 2, 512], f32, tag="s", bufs=2)
                        oo0 = oo[:, 0, :]
                    for h in range(h_local):
                        nc.tensor.matmul(
                            oo0,
                            lhsT=pend_ot[(qc, h)][:, ms * 128 : (ms + 1) * 128],
                            rhs=wo_sb[:, h, on * SQ : (on + 1) * SQ],
                            start=(h == 0),
                            stop=(h == h_local - 1),
                        )
                    # bf16 output halves evac + DMA cost; evacuate on DVE --
                    # ScalarE is saturated by exp in the surrounding chunk
                    out_t = attn.tile([128, SQ], bf16, tag="out_t", bufs=8)
                    nc.vector.tensor_copy(out_t, oo0)
                    nc.sync.dma_start(
                        out=out[
                            qc * SQ + ms * 128 : qc * SQ + (ms + 1) * 128,
                            on * SQ : (on + 1) * SQ,
                        ],
                        in_=out_t,
                    )

                # Software pipeline: the exp for a unit (pair of full k-blocks,
                # or half the diagonal trapezoid) is emitted right after its S
                # matmuls so ScalarE starts immediately; mask/AV/l consumption
                # runs one unit behind, so the PE always has the next unit's S
                # matmuls to chew on while the exp completes.
                def consume(st):
                    normalized = False
                    (segs, is_diag, p_sb, o_ps, l_ps, h, qc, n_kb) = st
                    if is_diag:
                        # triangular masks for the first 128 cols of each seg
                        nc.vector.tensor_mul(
                            p_sb[:, :, 0:KB], p_sb[:, :, 0:KB], m_sb
                        )
                    for plane, kb, w, qoff in segs:
                        rhs = p_sb[:, plane, 0:w]
                        start = kb == 0
                        stop = kb == n_kb - 1
                        nc.tensor.matmul(
                            o_ps[:, qoff:SQ],
                            lhsT=v_sb[:, kb, h * D : (h + 1) * D],
                            rhs=rhs,
                            start=start,
                            stop=stop,
                            skip_group_check=True,
                        )
                        nc.tensor.matmul(
                            l_ps[:, qoff:SQ],
                            lhsT=ones_sb,
                            rhs=rhs,
                            start=start,
                            stop=stop,
                            skip_group_check=True,
                        )
                    if segs[-1][1] == n_kb - 1:
                        # last unit of (h, qc): normalize ot = o_ps * (1/l),
                        # all on DVE (ScalarE has no throughput headroom):
                        # integer magic-number seed + one Newton step gives
                        # 1/l to ~0.1% -- plenty under the softmax noise.
                        y0 = attn.tile([128, SQ], f32, tag="linv", bufs=4)
                        nc.vector.scalar_tensor_tensor(
                            out=y0.bitcast(u32), in0=magic_sb, scalar=0.0,
                            in1=l_ps.bitcast(u32), op0=BYP, op1=SUB,
                        )
                        t = attn.tile([128, SQ], f32, tag="linv", bufs=4)
                        nc.vector.tensor_mul(t, l_ps, y0)
                        t2 = attn.tile([128, SQ], f32, tag="linv", bufs=4)
                        # -y1 = (l*y0 - 2) * y0
                        nc.vector.scalar_tensor_tensor(
                            out=t2, in0=t, scalar=2.0, in1=y0,
                            op0=SUB, op1=MUL,
                        )
                        ot = attn.tile([128, SQ], bf16, tag="ot", bufs=8)
                        # ot = (o_ps * -1) * (-y1)
                        nc.vector.scalar_tensor_tensor(
                            out=ot, in0=o_ps, scalar=-1.0, in1=t2,
                            op0=MUL, op1=MUL,
                        )
                        pend_ot[(qc, h)] = ot
                        normalized = True
                    return normalized

                pend = None
                pending_oproj = []
                # reverse chunk order: the PE-rich qc=3 runs first (nothing
                # pending to interleave), and the ScalarE-bound qc=0 runs
                # last with o_proj(1)'s matmuls soaking up the PE slack
                for qc in reversed(range(n_qc)):
                    n_kb = (qc + 1) * (SQ // KB)  # causal: blocks 0..n_kb-1
                    kbB = 4 * qc                  # first diagonal block
                    # the previous chunk's o_proj blocks are emitted in
                    # groups right after each normalize (which frees the
                    # "o"-ring slots the oo tiles reuse -- deadlock-free)
                    blocks_per_head = (
                        (len(pending_oproj) + h_local - 1) // h_local
                        if pending_oproj else 0
                    )
                    heads_done = 0
                    emitted = 0
                    for h in range(h_local):
                        o_ps = psum.tile([128, SQ], f32, tag="o", bufs=4)
                        l_ps = psum.tile([128, SQ], f32, tag="o", bufs=4)
                        # units: full pairs then the packed diagonal trapezoid
                        units = [
                            (False, [(p, 2 * kbp + p, 512, 0) for p in range(2)])
                            for kbp in range(2 * qc)
                        ]
                        units.append(
                            (True, [(0, kbB, 512, 0), (1, kbB + 1, 384, 128)])
                        )
                        units.append(
                            (True, [(0, kbB + 2, 256, 256), (1, kbB + 3, 128, 384)])
                        )
                        for is_diag, segs in units:
                            s_ps = psum.tile([128, 2, 512], f32, tag="s", bufs=2)
                            for plane, kb, w, qoff in segs:
                                nc.tensor.matmul(
                                    s_ps[:, plane, 0:w],
                                    lhsT=k_sb[:, h, kb * KB : (kb + 1) * KB],
                                    rhs=q_sb[:, h, qc * SQ + qoff : (qc + 1) * SQ],
                                    start=True,
                                    stop=True,
                                )
                            p_sb = attn.tile([128, 2, 512], bf16, tag="p_sb", bufs=4)
                            if segs[-1][2] <= 256:
                                # second diagonal half: only 2x256 cols live
                                nc.scalar.activation(
                                    p_sb[:, :, 0:256], s_ps[:, :, 0:256],
                                    EXP, scale=isqrt_d,
                                )
                            else:
                                nc.scalar.activation(p_sb, s_ps, EXP, scale=isqrt_d)
                            if pend is not None:
                                if consume(pend):
                                    heads_done += 1
                            pend = (segs, is_diag, p_sb, o_ps, l_ps, h, qc, n_kb)
                            want = min(
                                heads_done * blocks_per_head,
                                len(pending_oproj),
                            )
                            while emitted < want:
                                o_proj_block(*pending_oproj[emitted])
                                emitted += 1
                    # flush the pipeline at the chunk boundary
                    if pend is not None:
                        consume(pend)
                        pend = None
                    while emitted < len(pending_oproj):
                        o_proj_block(*pending_oproj[emitted])
                        emitted += 1
                    pending_oproj = [
                        (qc, ms, on)
                        for ms in range(n_ms)
                        for on in range(n_on)
                    ]
                # trailing o_proj for the last chunk: alternate PSUM rings
                # so evacuation of one block overlaps the next block's matmuls
                for i, blk in enumerate(pending_oproj):
                    o_proj_block(*blk, ring="o" if i % 2 else "s")

    # Finalize (assigns semaphore waits), then legalize: TRN2 instructions
    # accept only ONE sync wait each (EventSemaphore: two, InstISA: zero).
    nc.to_json_bytes()
    _legalize_waits(nc, mybir)
    return nc


def _legalize_waits(nc, mybir):
    """TRN2 instructions carry at most ONE sync wait (InstEventSemaphore:
    two; raw InstISA: none).  Peel excess waits onto event-semaphore
    instructions inserted immediately before, on the same engine sequencer
    (program order keeps the semantics)."""
    nfix = 0
    for f in nc.m.functions:
        for blk in f.blocks:
            insts = list(blk.instructions)
            out = []
            changed = False
            for inst in insts:
                si = getattr(inst, "sync_info", None)
                waits = list(si.on_wait) if si is not None and si.on_wait else []
                tname = type(inst).__name__
                limit = 2 if tname == "InstEventSemaphore" else (
                    0 if tname == "InstISA" else 1
                )
                if len(waits) > limit:
                    keep, excess = waits[:limit], waits[limit:]
                    for k in range(0, len(excess), 2):
                        es = mybir.InstEventSemaphore(
                            name=f"I-waitfix-{nfix}", ins=[], outs=[]
                        )
                        nfix += 1
                        es.engine = inst.engine
                        es.sync_info = mybir.SyncInfo(
                            on_wait=list(excess[k : k + 2]), on_update=[]
                        )
                        nc.register_instruction(es)
                        out.append(es)
                    inst.sync_info = mybir.SyncInfo(
                        on_wait=keep, on_update=list(si.on_update or [])
                    )
                    changed = True
                out.append(inst)
            if changed:
                blk.instructions = out
    return nfix


# ---------------------------------------------------------------------------
# Host-side input prep
# ---------------------------------------------------------------------------

def _rope_cache_np(seq, d):
    inv_freq = 1.0 / (ROPE_BASE ** (np.arange(0, d, 2, dtype=np.float32) / d))
    pos = np.arange(seq, dtype=np.float32)
    rot = pos[:, None] * inv_freq[None, :].astype(np.float32)
    theta = np.concatenate([rot, rot], axis=-1)  # [s, d]
    return np.cos(theta).astype(np.float32), np.sin(theta).astype(np.float32)


def _rot_matrix_np(d):
    """lhsT for rotate_half: (rotT.T @ q) == rotate_half(q)."""
    h = d // 2
    RT = np.zeros((d, d), dtype=np.float32)
    RT[np.arange(h) + h, np.arange(h)] = -1.0  # out[i] = -q[i+h], i < h
    RT[np.arange(h), np.arange(h) + h] = 1.0   # out[i] = q[i-h],  i >= h
    return RT


def _tri_masks_np():
    """Two copies of the 128x128 lower-triangle mask: m[j, kk, qq] = kk <= qq."""
    kk = np.arange(KB)[None, :, None]
    qq = np.arange(KB)[None, None, :]
    tri = (kk <= qq).astype(np.float32)       # [1, 128, 128]
    return np.broadcast_to(tri, (2, KB, KB))


def _swizzle_kc(a2d):
    """[n_kc*128, F] -> [128, n_kc*F] (partition-contiguous SBUF layout)."""
    n_kc = a2d.shape[0] // 128
    return np.ascontiguousarray(
        a2d.reshape(n_kc, 128, a2d.shape[1]).transpose(1, 0, 2).reshape(128, -1)
    )


def _swizzle_x(a2d):
    """[n_kc*128, S] -> [128, n_sc, n_kc, 512] -> [128, flat] seq-chunk-major."""
    n_kc = a2d.shape[0] // 128
    n_sc = a2d.shape[1] // SQ
    return np.ascontiguousarray(
        a2d.reshape(n_kc, 128, n_sc, SQ)
        .transpose(1, 2, 0, 3)
        .reshape(128, -1)
    )


def make_in_maps(hidden_states, Wq, Wk, Wv, Wo):
    import ml_dtypes

    bf = ml_dtypes.bfloat16
    cos, sin = _rope_cache_np(SEQ, D)
    cosT = np.ascontiguousarray(cos.T).astype(bf)
    sinT = np.ascontiguousarray(sin.T).astype(bf)
    rotT = _rot_matrix_np(D).astype(bf)
    mask = _tri_masks_np()  # [2, 128, 128]
    maskS = np.ascontiguousarray(
        mask.transpose(1, 0, 2).reshape(128, -1)
    ).astype(bf)
    ones = np.ones((128, 128), dtype=bf)

    in_maps = []
    for core in range(N_CORES):
        b = core // N_HGROUPS
        g = core % N_HGROUPS
        rs = slice(g * H_LOCAL * D, (g + 1) * H_LOCAL * D)
        in_maps.append(
            {
                "xS": _swizzle_x(hidden_states[b].T).astype(bf),
                "wqS": _swizzle_kc(Wq[rs, :].T).astype(bf),
                "wkS": _swizzle_kc(Wk[rs, :].T).astype(bf),
                "wvS": _swizzle_kc(Wv[rs, :].T).astype(bf),
                "woS": _swizzle_kc(Wo[:, rs].T).astype(bf),
                "cosT": cosT,
                "sinT": sinT,
                "rotT": rotT,
                "maskS": maskS,
                "ones": ones,
            }
        )
    return in_maps


def combine_outputs(results):
    """results: list of 8 dicts with 'out' [SEQ, HIDDEN] -> [BATCH, SEQ, HIDDEN]."""
    out = np.zeros((BATCH, SEQ, HIDDEN), dtype=np.float32)
    for core, r in enumerate(results):
        b = core // N_HGROUPS
        out[b] += np.asarray(r["out"], dtype=np.float32)
    return out


_CACHE = {}


def run_hw(inputs, trace=False, **kw):
    """Run on 8 NeuronCores; returns (output, BassKernelResults)."""
    from concourse.bass_utils import run_bass_kernel_spmd

    if "nc" not in _CACHE:
        _CACHE["nc"] = build_bass()
    nc = _CACHE["nc"]
    in_maps = make_in_maps(
        np.asarray(inputs["hidden_states"], dtype=np.float32),
        np.asarray(inputs["Wq"], dtype=np.float32),
        np.asarray(inputs["Wk"], dtype=np.float32),
        np.asarray(inputs["Wv"], dtype=np.float32),
        np.asarray(inputs["Wo"], dtype=np.float32),
    )
    res = run_bass_kernel_spmd(
        nc, in_maps, core_ids=list(range(N_CORES)), trace=trace, **kw
    )
    return combine_outputs(res.results), res


def kernel(hidden_states, Wq, Wk, Wv, Wo):
    out, _ = run_hw(
        {
            "hidden_states": hidden_states,
            "Wq": Wq,
            "Wk": Wk,
            "Wv": Wv,
            "Wo": Wo,
        }
    )
    return out
